# revision 1
# baseline (speedup 1.0000x reference)
"""Causal self-attention (B=8, T=1024, C=768, H=12, D=64) on 8 TRN2 NeuronCores.

Sharding: data-parallel over batch — core b handles batch element b. No
collectives. Host pre-transposes x to x^T[b] and pre-casts operands to bf16;
all matmuls run bf16 with fp32 PSUM accumulation.

Per-core algorithm:
  v = x Wv in [t, c] layout (x^T stationary); v bias folded into the y^T
  stage (exact: softmax rows sum to 1). q^T,k^T = (Wqkv^T x^T + b) in
  [c3, t] layout (weights stationary). Per head h, key-block j (128 keys):
  S^T = K_j Q^T in PSUM [keys, q] (causal: only q >= 128j columns; blocks
  pack into [128,1024] PSUM tiles as {j0},{j1,j7},{j2,j6},{j3,j5},{j4} so
  one ACT exp covers each tile, scale=1/8), triangular mask-multiply on
  diagonal 128x128 blocks. Per q-tile i: y'[q,65] = sum_j P_j^T.T @ [V_j|1]
  accumulated in PSUM (two heads x two i-steps share one PSUM bank); col 64
  is the softmax denominator. Normalize via per-partition reciprocal+scale,
  PE-transpose into a per-pair [128, 1024] bf16 PSUM strip (head parity in
  partition halves), one DVE pass per pair adds the v-bias and lands y^T in
  SBUF. out[t, c] = y^T.T @ Wproj + b_proj (bias via K=1 ones matmul).

Emission is software-pipelined (static per-engine order => head-of-line
blocking): cycle hp interleaves AV(hp) i-steps with qk(hp+1) half-chunks
and S(hp+1) groups so PE fills ACT-paced exp stalls; v tiles fill the S(0)
cold start.

PSUM budget (8 banks): big [128,1024]fp32 x2 (4, shared v/qk/S/o) +
y' [128,512]fp32 x2 (2, two heads x two i-steps packed) +
tr [128,1024]bf16 x2 (2).
"""

import numpy as np
import ml_dtypes

B, T, C = 8, 1024, 768
H, D = 12, 64
C3 = 3 * C
KC = C // 128          # 6 contraction chunks over c_in
TT = T // 128          # 8 t-tiles of 128
NPAIR = H // 2

BIG_BUFS = 3
SM_BUFS = 1
TR_BUFS = 1
PP_BUFS = 20           # 10 P segs live per pair (5 groups x 2 heads)

_BF16 = ml_dtypes.bfloat16

_compiled = {}


def _build():
    from concourse import bacc, mybir
    import concourse.tile as tile
    from concourse.masks import make_identity, make_upper_triangular

    fp32 = mybir.dt.float32
    bf16 = mybir.dt.bfloat16

    nc = bacc.Bacc("TRN2", target_bir_lowering=False, debug=False,
                   enable_asserts=True, num_devices=B)

    xT = nc.dram_tensor("xT", [C, T], bf16, kind="ExternalInput")
    wqkv = nc.dram_tensor("wqkv", [C, C3], bf16, kind="ExternalInput")
    wproj = nc.dram_tensor("wproj", [C, C], bf16, kind="ExternalInput")
    # b_qkv rearranged host-side to [128, 18]: col j holds b_qkv[128j:128j+128]
    bqkv = nc.dram_tensor("bqkv", [128, C3 // 128], fp32, kind="ExternalInput")
    bproj = nc.dram_tensor("bproj", [1, C], bf16, kind="ExternalInput")
    out = nc.dram_tensor("out", [T, C], fp32, kind="ExternalOutput")

    Exp = mybir.ActivationFunctionType.Exp
    # S-block packing: groups of (j, base column) sharing one [128,1024]
    # PSUM tile => one exp per tile. Bases keep each block inside the tile.
    GROUPS = [((4, 0),), ((3, 0), (5, 640)), ((2, 0), (6, 768)),
              ((1, 0), (7, 896)), ((0, 0),)]

    with tile.TileContext(nc) as tc:
        with (
            tc.tile_pool(name="const", bufs=1) as const,
            tc.tile_pool(name="pP", bufs=PP_BUFS) as pP,
            tc.tile_pool(name="small", bufs=6) as small,
            tc.tile_pool(name="osb", bufs=4) as osb,
            tc.tile_pool(name="ps_big", bufs=BIG_BUFS, space="PSUM") as ps_big,
            tc.tile_pool(name="ps_sm", bufs=SM_BUFS, space="PSUM") as ps_sm,
            tc.tile_pool(name="ps_tr", bufs=TR_BUFS, space="PSUM") as ps_tr,
        ):
            # ---- persistent SBUF loads ----
            xT_sb = []
            wqkv_sb = []
            wproj_sb = []
            bqkv_sb = const.tile([128, C3 // 128], fp32, tag="bqkv", name="bqkv")
            nc.sync.dma_start(bqkv_sb[:], bqkv[:, :])
            # pair-0 q/k weight slivers (c3 cols {0:128, 768:896}) as one
            # strided DMA per chunk so qk(0) can start ~5us in   [SP queue]
            for kc in range(KC):
                rows = slice(kc * 128, (kc + 1) * 128)
                t_ = const.tile([128, T], bf16, tag=f"xT{kc}", name=f"xT{kc}")
                xT_sb.append(t_)
                w_ = const.tile([128, C3], bf16, tag=f"wqkv{kc}", name=f"wqkv{kc}")
                wqkv_sb.append(w_)
                nc.sync.dma_start(
                    w_[:, 0:2 * C].rearrange("p (a b) -> p a b", a=2)[:, :, 0:128],
                    wqkv[rows, 0:2 * C].rearrange("p (a b) -> p a b", a=2)[:, :, 0:128],
                )
            # x^T via the ACT HWDGE queue (ACT is idle in the prologue)
            for kc in range(KC):
                rows = slice(kc * 128, (kc + 1) * 128)
                nc.scalar.dma_start(xT_sb[kc][:], xT[rows, :])
            # bulk weight streaming via GPSIMD SWDGE: v cols, then the rest
            for kc in range(KC):
                rows = slice(kc * 128, (kc + 1) * 128)
                nc.gpsimd.dma_start(wqkv_sb[kc][:, 2 * C:], wqkv[rows, 2 * C:])
            for kc in range(KC):
                rows = slice(kc * 128, (kc + 1) * 128)
                nc.gpsimd.dma_start(
                    w := wqkv_sb[kc][:, 128:128 + 2 * C].rearrange(
                        "p (a b) -> p a b", a=2)[:, :, 0:C - 128],
                    wqkv[rows, 128:128 + 2 * C].rearrange(
                        "p (a b) -> p a b", a=2)[:, :, 0:C - 128],
                )
            for kc in range(KC):
                rows = slice(kc * 128, (kc + 1) * 128)
                p_ = const.tile([128, C], bf16, tag=f"wproj{kc}", name=f"wproj{kc}")
                nc.gpsimd.dma_start(p_[:], wproj[rows, :])
                wproj_sb.append(p_)
            bproj_sb = const.tile([1, C], bf16, tag="bproj", name="bproj")
            nc.gpsimd.dma_start(bproj_sb[:], bproj[:, :])
            ones_sb = const.tile([1, 128], bf16, tag="ones", name="ones")
            nc.vector.memset(ones_sb[:], 1.0)
            ident_sb = const.tile([128, 128], bf16, tag="ident", name="ident")
            make_identity(nc, ident_sb[:])
            # keep columns m >= l (query >= key) on the diagonal block
            trimask_sb = const.tile([128, 128], bf16, tag="trimask", name="trimask")
            make_upper_triangular(nc, trimask_sb[:], val=1.0, diag=True)

            qkT_sb = [const.tile([128, T], bf16, tag=f"qkT{c3}", name=f"qkT{c3}")
                      for c3 in range(2 * KC)]
            # v packed [t, 12 heads x (64 + ones col)]
            v_sb = [const.tile([128, H, D + 1], bf16, tag=f"v{tt}", name=f"v{tt}")
                    for tt in range(TT)]
            yT_sb = [const.tile([128, T], bf16, tag=f"yT{kc}", name=f"yT{kc}")
                     for kc in range(KC)]

            def emit_v(tt):
                ps = ps_big.tile([128, 1024], fp32, tag="big", name="v_ps")
                for kc in range(KC):
                    nc.tensor.matmul(
                        ps[:, 0:512],
                        xT_sb[kc][:, tt * 128:(tt + 1) * 128],
                        wqkv_sb[kc][:, 2 * C:2 * C + 512],
                        start=(kc == 0), stop=(kc == KC - 1),
                    )
                    nc.tensor.matmul(
                        ps[:, 512:768],
                        xT_sb[kc][:, tt * 128:(tt + 1) * 128],
                        wqkv_sb[kc][:, 2 * C + 512:3 * C],
                        start=(kc == 0), stop=(kc == KC - 1),
                    )
                vv = v_sb[tt]
                nc.vector.tensor_copy(
                    vv[:, :, 0:D],
                    ps[:, 0:768].rearrange("p (h d) -> p h d", d=D),
                )
                nc.vector.memset(vv[:, :, D:D + 1], 1.0)

            def emit_qk_half(hp, which, tchunk):
                c3 = hp if which == "q" else KC + hp
                ps = ps_big.tile([128, 1024], fp32, tag="big", name="qk_ps")
                sl = slice(tchunk * 512, (tchunk + 1) * 512)
                for kc in range(KC):
                    nc.tensor.matmul(
                        ps[:, sl],
                        wqkv_sb[kc][:, c3 * 128:(c3 + 1) * 128],
                        xT_sb[kc][:, sl],
                        start=(kc == 0), stop=(kc == KC - 1),
                    )
                nc.vector.tensor_scalar_add(
                    qkT_sb[c3][:, sl], ps[:, sl], bqkv_sb[:, c3:c3 + 1],
                )

            def emit_S_group(hp, segs, grp):
                qT = qkT_sb[hp]
                kT = qkT_sb[KC + hp]
                for h in (2 * hp, 2 * hp + 1):
                    poff = 64 * (h % 2)
                    S = ps_big.tile([128, 1024], fp32, tag="big", name="S")
                    span = 0
                    for j, base in grp:
                        qs = 128 * j
                        w = T - qs
                        span = base + w
                        first = base + min(512 - base % 512, w) if base < 512 \
                            else base + w
                        for a, b_ in ((base, first), (first, base + w)):
                            if b_ <= a:
                                continue
                            nc.tensor.matmul(
                                S[:, a:b_],
                                kT[poff:poff + 64, qs:qs + 128],
                                qT[poff:poff + 64, qs + (a - base):qs + (b_ - base)],
                                start=True, stop=True,
                            )
                    P = pP.tile([128, 1024], bf16, tag="P", name="P")
                    nc.scalar.activation(P[:, 0:span], S[:, 0:span], Exp,
                                         scale=0.125)
                    for j, base in grp:
                        nc.vector.tensor_mul(P[:, base:base + 128],
                                             P[:, base:base + 128],
                                             trimask_sb[:])
                        segs[h][j] = (P, base)

            def emit_S_j4_pair(hp, segs):
                # both heads' j4 block (512 cols each) share one PSUM tile
                # and one exp: halves the ACT op overhead for this group
                qT = qkT_sb[hp]
                kT = qkT_sb[KC + hp]
                S = ps_big.tile([128, 1024], fp32, tag="big", name="S")
                for idx, h in enumerate((2 * hp, 2 * hp + 1)):
                    poff = 64 * (h % 2)
                    nc.tensor.matmul(
                        S[:, 512 * idx:512 * idx + 512],
                        kT[poff:poff + 64, 512:640],
                        qT[poff:poff + 64, 512:1024],
                        start=True, stop=True,
                    )
                P = pP.tile([128, 1024], bf16, tag="P", name="P")
                nc.scalar.activation(P[:], S[:], Exp, scale=0.125)
                for idx, h in enumerate((2 * hp, 2 * hp + 1)):
                    base = 512 * idx
                    nc.vector.tensor_mul(P[:, base:base + 128],
                                         P[:, base:base + 128],
                                         trimask_sb[:])
                    segs[h][4] = (P, base)

            def emit_AV_half(hp, segs, yns, i, y2, half):
                pair = (2 * hp, 2 * hp + 1)
                b0 = 256 * half
                for idx, h in enumerate(pair):
                    c0 = b0 + 128 * idx
                    for j in range(i + 1):
                        P, base = segs[h][j]
                        off = base + 128 * (i - j)
                        nc.tensor.matmul(
                            y2[:, c0:c0 + D + 1],
                            P[:, off:off + 128],
                            v_sb[j][:, h, :],
                            start=(j == 0), stop=(j == i),
                        )
                recip = small.tile([128, 2], fp32, tag="recip", name="recip")
                nc.vector.reciprocal(
                    recip[:],
                    y2[:].rearrange("p (g c) -> p g c", c=128)[:, 2 * half:2 * half + 2, D],
                )
                for idx, h in enumerate(pair):
                    c0 = b0 + 128 * idx
                    yn = small.tile([128, D], bf16, tag="yn", name="yn",
                                    bufs=18)
                    nc.vector.tensor_scalar_mul(yn[:], y2[:, c0:c0 + D],
                                                recip[:, idx:idx + 1])
                    yns.append((h, i, yn))

            def emit_yT(hp, trs):
                nc.vector.tensor_scalar_add(
                    yT_sb[hp][:],
                    trs[:],
                    bqkv_sb[:, 2 * KC + hp:2 * KC + hp + 1],
                )

            def new_segs():
                return {h: {} for h in range(H)}

            # ---- cold start: qk(0) first, then S(0) groups (j0-first so
            # AV(0,0) unblocks early) with 1:1 v backfill ----
            segs = {0: new_segs()}
            for which, tchunk in (("q", 0), ("k", 0), ("q", 1), ("k", 1)):
                emit_qk_half(0, which, tchunk)
            for g in range(4):
                emit_S_group(0, segs[0], GROUPS[4 - g])
                emit_v(g)
            emit_S_j4_pair(0, segs[0])
            emit_v(4)
            emit_v(5)

            # ---- pipelined cycles ----
            # per cycle: 8 AV i-steps; qk(hp+1) halves at steps 0,1,3; S(hp+1)
            # groups j0-first at steps 2,4,5,6,7 (j0 consumed first next cycle).
            qk_sched = {0: [("q", 0), ("k", 0)], 1: [("q", 1)], 3: [("k", 1)]}
            def emit_transpose_slice(trs, yns, sl):
                for h, i, yn in yns[sl]:
                    nc.tensor.transpose(
                        trs[64 * (h % 2):64 * (h % 2) + 64,
                            128 * i:128 * (i + 1)],
                        yn[:], ident_sb[:])

            prev_yns = None
            prev_trs = None
            for hp in range(NPAIR):
                nxt = hp + 1 < NPAIR
                if nxt:
                    segs[hp + 1] = new_segs()
                y2 = None
                yns = []
                trs = ps_tr.tile([128, 1024], bf16, tag="tr", name="tr")                     if prev_yns is not None else None
                for i in range(TT):
                    if i % 2 == 0:
                        y2 = ps_sm.tile([128, 512], fp32, tag="sm", name="y2")
                    emit_AV_half(hp, segs[hp], yns, i, y2, i % 2)
                    if hp == 0 and i in (3, 5):
                        emit_v(6 if i == 3 else 7)
                    if prev_yns is not None:
                        emit_transpose_slice(trs, prev_yns,
                                             slice(2 * i, 2 * i + 2))
                    if nxt:
                        for args in qk_sched.get(i, []):
                            emit_qk_half(hp + 1, *args)
                        gidx = {2: 4, 4: 3, 5: 2, 6: 1}.get(i)
                        if gidx is not None:
                            emit_S_group(hp + 1, segs[hp + 1], GROUPS[gidx])
                        elif i == 7:
                            emit_S_j4_pair(hp + 1, segs[hp + 1])
                if prev_yns is not None:
                    emit_yT(hp - 1, trs)
                prev_yns = yns
                segs.pop(hp)
            trs = ps_tr.tile([128, 1024], bf16, tag="tr", name="tr")
            emit_transpose_slice(trs, prev_yns, slice(0, 16))
            emit_yT(NPAIR - 1, trs)

            # ---- output projection ----
            for tt in range(TT):
                ps = ps_big.tile([128, 1024], fp32, tag="big", name="o_ps")
                for a, b_ in ((0, 512), (512, 768)):
                    for kc in range(KC):
                        nc.tensor.matmul(
                            ps[:, a:b_],
                            yT_sb[kc][:, tt * 128:(tt + 1) * 128],
                            wproj_sb[kc][:, a:b_],
                            start=(kc == 0), stop=False,
                        )
                    nc.tensor.matmul(
                        ps[:, a:b_], ones_sb[:], bproj_sb[:, a:b_],
                        start=False, stop=True,
                    )
                o = osb.tile([128, C], fp32, tag="o_sb", name="o_sb")
                nc.scalar.copy(o[:], ps[:, 0:768])
                nc.sync.dma_start(out[tt * 128:(tt + 1) * 128, :], o[:])

    nc.compile()
    return nc


def _prep_inputs(x, w_qkv, b_qkv, w_proj, b_proj):
    wqkv_bf = np.ascontiguousarray(w_qkv.astype(_BF16))
    wproj_bf = np.ascontiguousarray(w_proj.astype(_BF16))
    bqkv_pc = np.ascontiguousarray(b_qkv.astype(np.float32).reshape(C3 // 128, 128).T)
    bproj_bf = np.ascontiguousarray(b_proj.astype(_BF16).reshape(1, C))
    in_maps = []
    for b in range(B):
        xTb = np.ascontiguousarray(x[b].astype(_BF16).T)
        in_maps.append({
            "xT": xTb,
            "wqkv": wqkv_bf,
            "wproj": wproj_bf,
            "bqkv": bqkv_pc,
            "bproj": bproj_bf,
        })
    return in_maps


def _run(inputs, trace=False):
    from concourse.bass_utils import run_bass_kernel_spmd

    if "nc" not in _compiled:
        _compiled["nc"] = _build()
    nc = _compiled["nc"]
    in_maps = _prep_inputs(inputs["x"], inputs["w_qkv"], inputs["b_qkv"],
                           inputs["w_proj"], inputs["b_proj"])
    res = run_bass_kernel_spmd(nc, in_maps, list(range(B)), trace=trace)
    outs = np.stack([np.asarray(res.results[b]["out"]) for b in range(B)])
    return outs.astype(np.float32), res


def kernel(x, w_qkv, b_qkv, w_proj, b_proj):
    outs, _ = _run(dict(x=x, w_qkv=w_qkv, b_qkv=b_qkv,
                        w_proj=w_proj, b_proj=b_proj))
    return outs



# revision 27
# speedup vs baseline: 1.0689x; 1.0689x over previous
"""Causal self-attention (B=8, T=1024, C=768, H=12, D=64) on 8 TRN2 NeuronCores.

Sharding: data-parallel over batch — core b handles batch element b. No
collectives. Host pre-transposes x to x^T[b] and pre-casts operands to bf16;
all matmuls run bf16 with fp32 PSUM accumulation.

Per-core algorithm:
  v = x Wv in [t, c] layout (x^T stationary); v bias folded into the y^T
  stage (exact: softmax rows sum to 1). q^T,k^T = (Wqkv^T x^T + b) in
  [c3, t] layout (weights stationary). Per head h, key-block j (128 keys):
  S^T = K_j Q^T in PSUM [keys, q] (causal: only q >= 128j columns; blocks
  pack into [128,1024] PSUM tiles as {j0},{j1,j7},{j2,j6},{j3,j5},{j4} so
  one ACT exp covers each tile, scale=1/8), triangular mask-multiply on
  diagonal 128x128 blocks. Per q-tile i: y'[q,65] = sum_j P_j^T.T @ [V_j|1]
  accumulated in PSUM (two heads x two i-steps share one PSUM bank); col 64
  is the softmax denominator. Normalize via per-partition reciprocal+scale,
  PE-transpose into a per-pair [128, 1024] bf16 PSUM strip (head parity in
  partition halves), one DVE pass per pair adds the v-bias and lands y^T in
  SBUF. out[t, c] = y^T.T @ Wproj + b_proj (bias via K=1 ones matmul).

Emission is software-pipelined (static per-engine order => head-of-line
blocking): cycle hp interleaves AV(hp) i-steps with qk(hp+1) half-chunks
and S(hp+1) groups so PE fills ACT-paced exp stalls; v tiles fill the S(0)
cold start.

PSUM budget (8 banks): big [128,1024]fp32 x2 (4, shared v/qk/S/o) +
y' [128,512]fp32 x2 (2, two heads x two i-steps packed) +
tr [128,1024]bf16 x2 (2).
"""

import numpy as np
import ml_dtypes

B, T, C = 8, 1024, 768
H, D = 12, 64
C3 = 3 * C
KC = C // 128          # 6 contraction chunks over c_in
TT = T // 128          # 8 t-tiles of 128
NPAIR = H // 2

BIG_BUFS = 3
SM_BUFS = 1
TR_BUFS = 1
PP_BUFS = 20           # 10 P segs live per pair (5 groups x 2 heads)

_BF16 = ml_dtypes.bfloat16

_compiled = {}


def _build():
    from concourse import bacc, mybir
    import concourse.tile as tile
    from concourse.masks import make_identity, make_upper_triangular

    fp32 = mybir.dt.float32
    bf16 = mybir.dt.bfloat16

    nc = bacc.Bacc("TRN2", target_bir_lowering=False, debug=False,
                   enable_asserts=True, num_devices=B)

    xT = nc.dram_tensor("xT", [C, T], bf16, kind="ExternalInput")
    wqkv = nc.dram_tensor("wqkv", [C, C3], bf16, kind="ExternalInput")
    wproj = nc.dram_tensor("wproj", [C, C], bf16, kind="ExternalInput")
    # b_qkv rearranged host-side to [128, 18]: col j holds b_qkv[128j:128j+128]
    bqkv = nc.dram_tensor("bqkv", [128, C3 // 128], fp32, kind="ExternalInput")
    # b_proj rearranged host-side to [128, 6]: col j holds b_proj[128j:128j+128]
    bproj = nc.dram_tensor("bproj", [128, C // 128], fp32, kind="ExternalInput")
    # out is produced transposed [C, T] in bf16; host transposes + casts back
    out = nc.dram_tensor("out", [C, T], bf16, kind="ExternalOutput")

    Exp = mybir.ActivationFunctionType.Exp
    # S-block packing: groups of (j, base column) sharing one [128,1024]
    # PSUM tile => one exp per tile. Bases keep each block inside the tile.
    GROUPS = [((4, 0),), ((3, 0), (5, 640)), ((2, 0), (6, 768)),
              ((1, 0), (7, 896)), ((0, 0),)]

    with tile.TileContext(nc) as tc:
        with (
            tc.tile_pool(name="const", bufs=1) as const,
            tc.tile_pool(name="pP", bufs=PP_BUFS) as pP,
            tc.tile_pool(name="small", bufs=6) as small,
            tc.tile_pool(name="osb", bufs=4) as osb,
            tc.tile_pool(name="ps_big", bufs=BIG_BUFS, space="PSUM") as ps_big,
            tc.tile_pool(name="ps_sm", bufs=SM_BUFS, space="PSUM") as ps_sm,
            tc.tile_pool(name="ps_tr", bufs=TR_BUFS, space="PSUM") as ps_tr,
        ):
            # ---- persistent SBUF loads ----
            # All input streaming via the two HWDGE queues (SP + ACT), in
            # need-order: hp0+1 q/k sliver cols + x^T first (qk(0)), then
            # v cols, hp2-5 q/k cols, wproj.  DMA_ENGINES is a serial
            # resource: issue order IS the arrival order.
            bqkv_sb = const.tile([128, C3 // 128], fp32, tag="bqkv", name="bqkv")
            wq_big = const.tile([128, KC, C3], bf16, tag="wqkv", name="wqkv")
            wqkv_sb = [wq_big[:, kc] for kc in range(KC)]
            wqkv_src = wqkv.rearrange("(k p) c -> p k c", k=KC)
            xT_sb = [const.tile([128, T], bf16, tag=f"xT{kc}", name=f"xT{kc}")
                     for kc in range(KC)]
            nc.sync.dma_start(wq_big[:, :, 0:256], wqkv_src[:, :, 0:256])
            nc.scalar.dma_start(xT_sb[0][:], xT[0:128, :])
            nc.sync.dma_start(wq_big[:, :, C:C + 256], wqkv_src[:, :, C:C + 256])
            nc.scalar.dma_start(xT_sb[1][:], xT[128:256, :])
            nc.sync.dma_start(xT_sb[2][:], xT[256:384, :])
            nc.scalar.dma_start(xT_sb[3][:], xT[384:512, :])
            nc.sync.dma_start(xT_sb[4][:], xT[512:640, :])
            nc.scalar.dma_start(xT_sb[5][:], xT[640:768, :])
            nc.sync.dma_start(bqkv_sb[:], bqkv[:, :])
            nc.sync.dma_start(wq_big[:, :, 2 * C:], wqkv_src[:, :, 2 * C:])
            nc.sync.dma_start(wq_big[:, :, 256:C], wqkv_src[:, :, 256:C])
            nc.sync.dma_start(wq_big[:, :, C + 256:2 * C],
                              wqkv_src[:, :, C + 256:2 * C])
            wproj_big = const.tile([128, KC, C], bf16, tag="wproj", name="wproj")
            wproj_sb = [wproj_big[:, kc] for kc in range(KC)]
            nc.sync.dma_start(
                wproj_big[:],
                wproj.rearrange("(k p) c -> p k c", k=KC),
            )
            bproj_sb = const.tile([128, C // 128], fp32, tag="bproj", name="bproj")
            nc.sync.dma_start(bproj_sb[:], bproj[:, :])
            ident_sb = const.tile([128, 128], bf16, tag="ident", name="ident")
            make_identity(nc, ident_sb[:])
            # keep columns m >= l (query >= key) on the diagonal block
            trimask_sb = const.tile([128, 128], bf16, tag="trimask", name="trimask")
            make_upper_triangular(nc, trimask_sb[:], val=1.0, diag=True)

            qkT_sb = [const.tile([128, T], bf16, tag=f"qkT{c3}", name=f"qkT{c3}")
                      for c3 in range(2 * KC)]
            # v packed [t, 12 heads x (64 + ones col)]
            v_sb = [const.tile([128, H, D + 1], bf16, tag=f"v{tt}", name=f"v{tt}")
                    for tt in range(TT)]
            yT_sb = [const.tile([128, T], bf16, tag=f"yT{kc}", name=f"yT{kc}")
                     for kc in range(KC)]

            def emit_v(tt):
                ps = ps_big.tile([128, 1024], fp32, tag="big", name="v_ps")
                for kc in range(KC):
                    nc.tensor.matmul(
                        ps[:, 0:512],
                        xT_sb[kc][:, tt * 128:(tt + 1) * 128],
                        wqkv_sb[kc][:, 2 * C:2 * C + 512],
                        start=(kc == 0), stop=(kc == KC - 1),
                    )
                    nc.tensor.matmul(
                        ps[:, 512:768],
                        xT_sb[kc][:, tt * 128:(tt + 1) * 128],
                        wqkv_sb[kc][:, 2 * C + 512:3 * C],
                        start=(kc == 0), stop=(kc == KC - 1),
                    )
                vv = v_sb[tt]
                nc.vector.tensor_copy(
                    vv[:, :, 0:D],
                    ps[:, 0:768].rearrange("p (h d) -> p h d", d=D),
                )
                nc.vector.memset(vv[:, :, D:D + 1], 1.0)

            Identity = mybir.ActivationFunctionType.Identity

            def emit_qk_half(hp, which, tchunk, on_act=False):
                c3 = hp if which == "q" else KC + hp
                ps = ps_big.tile([128, 1024], fp32, tag="big", name="qk_ps")
                sl = slice(tchunk * 512, (tchunk + 1) * 512)
                for kc in range(KC):
                    nc.tensor.matmul(
                        ps[:, sl],
                        wqkv_sb[kc][:, c3 * 128:(c3 + 1) * 128],
                        xT_sb[kc][:, sl],
                        start=(kc == 0), stop=(kc == KC - 1),
                    )
                if on_act:
                    nc.scalar.activation(qkT_sb[c3][:, sl], ps[:, sl],
                                         Identity, bias=bqkv_sb[:, c3:c3 + 1])
                else:
                    nc.vector.tensor_scalar_add(
                        qkT_sb[c3][:, sl], ps[:, sl], bqkv_sb[:, c3:c3 + 1],
                    )

            def emit_S_group(hp, segs, grp):
                qT = qkT_sb[hp]
                kT = qkT_sb[KC + hp]
                for h in (2 * hp, 2 * hp + 1):
                    poff = 64 * (h % 2)
                    S = ps_big.tile([128, 1024], fp32, tag="big", name="S")
                    span = 0
                    for j, base in grp:
                        qs = 128 * j
                        w = T - qs
                        span = base + w
                        first = base + min(512 - base % 512, w) if base < 512 \
                            else base + w
                        for a, b_ in ((base, first), (first, base + w)):
                            if b_ <= a:
                                continue
                            nc.tensor.matmul(
                                S[:, a:b_],
                                kT[poff:poff + 64, qs:qs + 128],
                                qT[poff:poff + 64, qs + (a - base):qs + (b_ - base)],
                                start=True, stop=True,
                            )
                    P = pP.tile([128, 1024], bf16, tag="P", name="P")
                    nc.scalar.activation(P[:, 0:span], S[:, 0:span], Exp,
                                         scale=0.125)
                    for j, base in grp:
                        nc.gpsimd.tensor_mul(P[:, base:base + 128],
                                             P[:, base:base + 128],
                                             trimask_sb[:])
                        segs[h][j] = (P, base)

            def emit_S_j4_pair(hp, segs):
                # both heads' j4 block (512 cols each) share one PSUM tile
                # and one exp: halves the ACT op overhead for this group
                qT = qkT_sb[hp]
                kT = qkT_sb[KC + hp]
                S = ps_big.tile([128, 1024], fp32, tag="big", name="S")
                for idx, h in enumerate((2 * hp, 2 * hp + 1)):
                    poff = 64 * (h % 2)
                    nc.tensor.matmul(
                        S[:, 512 * idx:512 * idx + 512],
                        kT[poff:poff + 64, 512:640],
                        qT[poff:poff + 64, 512:1024],
                        start=True, stop=True,
                    )
                P = pP.tile([128, 1024], bf16, tag="P", name="P")
                nc.scalar.activation(P[:], S[:], Exp, scale=0.125)
                for idx, h in enumerate((2 * hp, 2 * hp + 1)):
                    base = 512 * idx
                    nc.gpsimd.tensor_mul(P[:, base:base + 128],
                                         P[:, base:base + 128],
                                         trimask_sb[:])
                    segs[h][4] = (P, base)

            def emit_AV_half(hp, segs, yns, i, y2, half):
                pair = (2 * hp, 2 * hp + 1)
                b0 = 256 * half
                for idx, h in enumerate(pair):
                    c0 = b0 + 128 * idx
                    for j in range(i + 1):
                        P, base = segs[h][j]
                        off = base + 128 * (i - j)
                        nc.tensor.matmul(
                            y2[:, c0:c0 + D + 1],
                            P[:, off:off + 128],
                            v_sb[j][:, h, :],
                            start=(j == 0), stop=(j == i),
                        )
                recip = small.tile([128, 2], fp32, tag="recip", name="recip")
                nc.vector.reciprocal(
                    recip[:],
                    y2[:].rearrange("p (g c) -> p g c", c=128)[:, 2 * half:2 * half + 2, D],
                )
                # both heads' normalized y packed [128, 128] -> one transpose
                yn = small.tile([128, 2 * D], bf16, tag="yn", name="yn",
                                bufs=10)
                for idx, h in enumerate(pair):
                    c0 = b0 + 128 * idx
                    nc.vector.tensor_scalar_mul(yn[:, idx * D:(idx + 1) * D],
                                                y2[:, c0:c0 + D],
                                                recip[:, idx:idx + 1])
                yns.append((i, yn))

            def emit_yT(hp, trs):
                nc.vector.tensor_scalar_add(
                    yT_sb[hp][:],
                    trs[:],
                    bqkv_sb[:, 2 * KC + hp:2 * KC + hp + 1],
                )

            def new_segs():
                return {h: {} for h in range(H)}

            # ---- cold start: qk(0) first, then S(0) groups (j0-first so
            # AV(0,0) unblocks early) with 1:1 v backfill ----
            segs = {0: new_segs()}
            for which, tchunk in (("q", 0), ("q", 1), ("k", 0), ("k", 1)):
                emit_qk_half(0, which, tchunk, on_act=True)
            for g in range(4):
                emit_S_group(0, segs[0], GROUPS[4 - g])
                emit_v(g)
            emit_S_j4_pair(0, segs[0])
            emit_v(4)
            emit_v(5)

            # ---- pipelined cycles ----
            # per cycle: 8 AV i-steps; qk(hp+1) halves at steps 0,1,3; S(hp+1)
            # groups j0-first at steps 2,4,5,6,7 (j0 consumed first next cycle).
            qk_sched = {0: [("q", 0), ("k", 0)], 1: [("q", 1)], 2: [("k", 1)]}
            def emit_transpose_slice(trs, yns, sl):
                for i, yn in yns[sl]:
                    nc.tensor.transpose(trs[:, 128 * i:128 * (i + 1)],
                                        yn[:], ident_sb[:])

            def emit_proj_mms(ps, cc, a, b_, kcs, stop_kc):
                for kc in kcs:
                    nc.tensor.matmul(
                        ps[:, a:b_],
                        wproj_sb[kc][:, cc * 128:(cc + 1) * 128],
                        yT_sb[kc][:, a:b_],
                        start=(kc == 0), stop=(kc == stop_kc),
                    )

            def emit_proj_out(ps, cc):
                o = osb.tile([128, T], bf16, tag="o_sb", name="o_sb")
                nc.scalar.activation(o[:], ps[:], Identity,
                                     bias=bproj_sb[:, cc:cc + 1], scale=1.0)
                nc.sync.dma_start(out[cc * 128:(cc + 1) * 128, :], o[:])

            prev_yns = None
            proj_ps = {}
            for hp in range(NPAIR):
                nxt = hp + 1 < NPAIR
                last = not nxt
                if nxt:
                    segs[hp + 1] = new_segs()
                y2 = None
                yns = []
                trs = ps_tr.tile([128, 1024], bf16, tag="tr", name="tr")                     if prev_yns is not None else None
                for i in range(TT):
                    if i % 2 == 0:
                        if i == 0 and hp > 0:
                            # borrow an idle big-pool bank so AV(i=0,1) need
                            # not wait for the previous pair's y2 normalize
                            y2 = ps_big.tile([128, 1024], fp32, tag="big",
                                             name="y2big")[:, 0:512]
                        else:
                            y2 = ps_sm.tile([128, 512], fp32, tag="sm",
                                            name="y2")
                    emit_AV_half(hp, segs[hp], yns, i, y2, i % 2)
                    if hp == 0 and i in (3, 5):
                        emit_v(6 if i == 3 else 7)
                    if prev_yns is not None:
                        emit_transpose_slice(trs, prev_yns,
                                             slice(i, i + 1))
                    if nxt:
                        for args in qk_sched.get(i, []):
                            emit_qk_half(hp + 1, *args)
                        gidx = {2: 4, 3: 3, 4: 2, 5: 1}.get(i)
                        if gidx is not None:
                            emit_S_group(hp + 1, segs[hp + 1], GROUPS[gidx])
                        elif i == 6:
                            emit_S_j4_pair(hp + 1, segs[hp + 1])
                    elif 1 <= i <= 6:
                        # last pair: park cc0-2 projection partials (kc 0-3,
                        # yT[0..3] are final) in the now-idle big pool
                        cc, half = divmod(i - 1, 2)
                        if half == 0:
                            proj_ps[cc] = ps_big.tile([128, 1024], fp32,
                                                      tag="big", name="o_ps")
                        emit_proj_mms(proj_ps[cc], cc, 512 * half,
                                      512 * (half + 1), range(4), None)
                if prev_yns is not None:
                    emit_yT(hp - 1, trs)
                prev_yns = yns
                segs.pop(hp)
            # yT[4] just landed: extend parked partials with kc=4
            for cc in range(3):
                for a, b_ in ((0, 512), (512, 1024)):
                    emit_proj_mms(proj_ps[cc], cc, a, b_, (4,), None)
            trs = ps_tr.tile([128, 1024], bf16, tag="tr", name="tr")
            emit_transpose_slice(trs, prev_yns, slice(0, 8))
            emit_yT(NPAIR - 1, trs)

            # ---- projection endgame: finish parked cc0-2, then cc3-5 ----
            for cc in range(KC):
                if cc < 3:
                    ps = proj_ps.pop(cc)
                    for a, b_ in ((0, 512), (512, 1024)):
                        emit_proj_mms(ps, cc, a, b_, (5,), 5)
                else:
                    ps = ps_big.tile([128, 1024], fp32, tag="big", name="o_ps")
                    for a, b_ in ((0, 512), (512, 1024)):
                        emit_proj_mms(ps, cc, a, b_, range(KC), KC - 1)
                emit_proj_out(ps, cc)

    nc.compile()
    return nc


def _prep_inputs(x, w_qkv, b_qkv, w_proj, b_proj):
    wqkv_bf = np.ascontiguousarray(w_qkv.astype(_BF16))
    wproj_bf = np.ascontiguousarray(w_proj.astype(_BF16))
    bqkv_pc = np.ascontiguousarray(b_qkv.astype(np.float32).reshape(C3 // 128, 128).T)
    bproj_pc = np.ascontiguousarray(b_proj.astype(np.float32).reshape(C // 128, 128).T)
    in_maps = []
    for b in range(B):
        xTb = np.ascontiguousarray(x[b].astype(_BF16).T)
        in_maps.append({
            "xT": xTb,
            "wqkv": wqkv_bf,
            "wproj": wproj_bf,
            "bqkv": bqkv_pc,
            "bproj": bproj_pc,
        })
    return in_maps


def _run(inputs, trace=False):
    from concourse.bass_utils import run_bass_kernel_spmd

    if "nc" not in _compiled:
        _compiled["nc"] = _build()
    nc = _compiled["nc"]
    in_maps = _prep_inputs(inputs["x"], inputs["w_qkv"], inputs["b_qkv"],
                           inputs["w_proj"], inputs["b_proj"])
    res = run_bass_kernel_spmd(nc, in_maps, list(range(B)), trace=trace)
    outs = np.stack([np.asarray(res.results[b]["out"]).T for b in range(B)])
    return outs.astype(np.float32), res


def kernel(x, w_qkv, b_qkv, w_proj, b_proj):
    outs, _ = _run(dict(x=x, w_qkv=w_qkv, b_qkv=b_qkv,
                        w_proj=w_proj, b_proj=b_proj))
    return outs

